# revision 10
# baseline (speedup 1.0000x reference)
"""Trainium2 Bass kernel for nn_AttentionLayer (sparse_attention, 8-core head-parallel).

Reference computation (B=4, S=16, H=16, D=128, HID=2048, P=8192):
    qkv = x @ w_qkv + b_qkv ; split into q,k,v
    k_full = concat(cached_k broadcast over batch, new k)   # [B,H,P+S,D]
    out = softmax(q @ k_full^T / sqrt(D)) @ v_full
    y = out @ w_proj + b_proj

Sharding: tensor-parallel over heads. Each of the 8 cores owns 2 heads:
column-sharded w_qkv/b_qkv (its heads' q,k,v columns), the head slice of the
KV cache, and the row slice of w_proj. Each core emits a partial y
[64, 2048]; the unshard step sums the 8 partials and adds b_proj (row-parallel
linear with host-side reduction).

Device-side layout choices (prepared on host during the shard step):
  - TensorEngine-facing tensors are shipped/computed in bf16 (KV cache,
    weights, x, exp(scores)); all matmul accumulation, softmax input, the
    denominators and the output stay f32. Emulated end-to-end rel err vs the
    f32 reference is ~3e-3 (tolerance 2e-2).
  - x (pre-transposed k-major), the qkv weight shard (k-major) and the
    new-token block-diagonal mask ship as ONE contiguous dram param -> one
    3.4MB DMA covers everything the projection needs.
  - qkv projection computed in natural layout [64 tok, 768] (32 matmuls),
    with the bias added via a ones-row rank-1 matmul; q^T/k^T then produced
    by 4 PE transposes.
  - cached_k passed per head as K^T [D=128, P] in slab-contiguous form: each
    [128, 4096] bf16 slab is one contiguous 1MB DMA whose [128,128] slices
    are directly the stationary operand of the scores^T matmul.
  - cached_v passed with both heads interleaved [P, 129+129]: per-head 128
    value columns plus a constant ones column. Accumulating exp(scores^T)^T @
    [V | 1] yields the attention numerator AND softmax denominator in one
    matmul (scores are O(5) here, so exp needs no max-subtraction in f32).
  - 1/sqrt(D) is folded into the q columns of w_qkv/b_qkv.
  - New-token scores use the block-diagonal mask (queries attend only their
    own batch's 16 new keys), multiplied after exp -> exact zeros off-block.
  - scores^T for 4 chunks x 2 heads are packed into one [128, 512] PSUM bank
    so a single ACT instruction computes exp for all 8 score tiles.
"""

import math

import numpy as np
import ml_dtypes

import concourse.bass as bass
import concourse.mybir as mybir
import concourse.tile as tile
from concourse import bacc
from concourse.bass_utils import run_bass_kernel_spmd
from concourse.masks import make_identity

FP = mybir.dt.float32
BF = mybir.dt.bfloat16
NPBF = ml_dtypes.bfloat16
AFT = mybir.ActivationFunctionType

B, S, H, D = 4, 16, 16, 128
HID = H * D            # 2048
P = 8192               # cached prefix length
NQ = B * S             # 64 query tokens
NCORES = 8
HPC = H // NCORES      # heads per core = 2

NCHUNK = P // 128      # 64 cache chunks of 128 keys
GRP = 4                # chunks whose scores share one PSUM bank / one exp
NGRP = NCHUNK // GRP   # 16
KSLAB = 4096           # seq per K-slab DMA (32 chunks, 1MB bf16)
VSLAB = 16             # chunks per V-slab DMA (1.03MB bf16)
VW = D + 1             # 129: V columns + ones column

# combined xt|wqkv|mask param column offsets (bf16)
XT_C = 16 * NQ                 # 1024
WQ_C = 16 * 6 * 128            # 12288
MS_C = NQ                      # 64
WX_COLS = XT_C + WQ_C + MS_C   # 13376

_nc_cache = None


def _build_nc(reps=1, loop=None):
    nc = bacc.Bacc("TRN2", target_bir_lowering=False, debug=False,
                   num_devices=NCORES)

    wx_d = nc.declare_dram_parameter("wx", [128, WX_COLS], BF, isOutput=False)
    bqn_d = nc.declare_dram_parameter("bqn", [1, 768], BF, isOutput=False)
    kt_d = nc.declare_dram_parameter("kt", [HPC * P // KSLAB, 128, KSLAB], BF, isOutput=False)
    vb_d = nc.declare_dram_parameter("vb", [NCHUNK // VSLAB, 128, VSLAB * 2 * VW], BF, isOutput=False)
    wp_d = nc.declare_dram_parameter("wp", [128, HPC * HID], BF, isOutput=False)
    out_d = nc.declare_dram_parameter("out", [NQ, HID], FP, isOutput=True)

    with tile.TileContext(nc) as tc:
        with (
            tc.tile_pool(name="const", bufs=1) as constp,
            tc.tile_pool(name="wx", bufs=2) as wxp,
            tc.tile_pool(name="wproj", bufs=2) as wpp,
            tc.tile_pool(name="kslab", bufs=4) as kp,
            tc.tile_pool(name="vslab", bufs=3) as vp,
            tc.tile_pool(name="pt", bufs=4) as ptp,
            tc.tile_pool(name="small", bufs=4) as smallp,
            tc.tile_pool(name="ps_s", bufs=3, space="PSUM") as pssp,
            tc.tile_pool(name="ps_acc", bufs=2, space="PSUM") as paccp,
            tc.tile_pool(name="ps_gp", bufs=2, space="PSUM") as pgpp,
            tc.tile_pool(name="ps_misc", bufs=1, space="PSUM") as pmiscp,
        ):
            ident = constp.tile([128, 128], BF, tag="ident")
            make_identity(nc, ident[:])
            ones1 = constp.tile([1, NQ], BF, tag="ones1")
            nc.vector.memset(ones1[:], 1.0)

            def emit(r):
                # ---- loads ----
                wx = wxp.tile([128, WX_COLS], BF, tag="wx", name=f"wx{r}")
                nc.sync.dma_start(wx[:], wx_d[:])
                bqn = constp.tile([1, 768], BF, tag="bqn", name=f"bqn{r}")
                nc.sync.dma_start(bqn[:], bqn_d[:])
                kslab_per_head = P // KSLAB
                k_sb = [None, None]
                for h in range(HPC):
                    k_sb[h] = kp.tile([128, KSLAB], BF, tag="k", name=f"k0_{h}{r}")
                    nc.sync.dma_start(k_sb[h][:], kt_d[h * kslab_per_head])
                v_sb = vp.tile([128, VSLAB * 2 * VW], BF, tag="v", name=f"v0{r}")
                nc.sync.dma_start(v_sb[:], vb_d[0])

                xt = wx[:, 0:XT_C]
                msk = wx[0:NQ, XT_C + WQ_C:XT_C + WQ_C + MS_C]

                # ---- qkv projection, natural layout [64, 768] ----
                ps_a = pgpp.tile([NQ, 512], FP, tag="gp", name=f"qkva{r}")
                ps_b = pgpp.tile([NQ, 256], FP, tag="gp", name=f"qkvb{r}")
                for t in range(16):
                    w0 = XT_C + t * 768
                    nc.tensor.matmul(ps_a[:], lhsT=xt[:, t * NQ:(t + 1) * NQ],
                                     rhs=wx[:, w0:w0 + 512],
                                     start=(t == 0), stop=False)
                    nc.tensor.matmul(ps_b[:], lhsT=xt[:, t * NQ:(t + 1) * NQ],
                                     rhs=wx[:, w0 + 512:w0 + 768],
                                     start=(t == 0), stop=False)
                nc.tensor.matmul(ps_a[:], lhsT=ones1[:], rhs=bqn[:, 0:512],
                                 start=False, stop=True)
                nc.tensor.matmul(ps_b[:], lhsT=ones1[:], rhs=bqn[:, 512:768],
                                 start=False, stop=True)
                qn = constp.tile([NQ, 768], BF, tag="qn", name=f"qn{r}")
                nc.scalar.activation(qn[:, 0:512], ps_a[:], AFT.Copy)
                nc.scalar.activation(qn[:, 512:768], ps_b[:], AFT.Copy)

                # q^T, k^T per head via PE transpose
                qT = []
                kT = []
                for m in range(4):
                    t_ps = pmiscp.tile([128, NQ], BF, tag="misc", name=f"tps{m}{r}")
                    nc.tensor.transpose(t_ps[:], qn[:, m * 128:(m + 1) * 128],
                                        ident[0:NQ, 0:NQ])
                    t_sb = smallp.tile([128, NQ], BF, tag="qkT", name=f"qkT{m}{r}")
                    nc.vector.tensor_copy(t_sb[:], t_ps[:])
                    (qT if m < 2 else kT).append(t_sb)

                # ---- new-token attention pieces (tiny) ----
                vnew = []
                pnew = []
                for h in range(HPC):
                    vn = constp.tile([NQ, VW], BF, tag=f"vnew{h}", name=f"vnew{h}{r}")
                    nc.vector.tensor_copy(vn[:, 0:128],
                                          qn[:, 512 + h * 128:512 + (h + 1) * 128])
                    nc.vector.memset(vn[:, 128:129], 1.0)
                    vnew.append(vn)

                    sn_ps = pmiscp.tile([NQ, NQ], FP, tag="misc", name=f"snps{h}{r}")
                    nc.tensor.matmul(sn_ps[:], lhsT=kT[h][:], rhs=qT[h][:],
                                     start=True, stop=True)
                    pn = constp.tile([NQ, NQ], BF, tag=f"pn{h}", name=f"pn{h}{r}")
                    nc.scalar.activation(pn[:], sn_ps[:], AFT.Exp)
                    pnm = constp.tile([NQ, NQ], BF, tag=f"pnm{h}", name=f"pnm{h}{r}")
                    nc.vector.tensor_mul(pnm[:], pn[:], msk)
                    pnew.append(pnm)

                # ---- w_proj load (needed only at the tail) ----
                wp_sb = wpp.tile([128, HPC * HID], BF, tag="wp", name=f"wp{r}")
                nc.sync.dma_start(wp_sb[:], wp_d[:])

                # ---- main cache sweep, both heads interleaved ----
                accs = [paccp.tile([NQ, VW], FP, tag="acc", name=f"acc{i}{r}")
                        for i in range(HPC)]
                for g in range(NGRP):
                    c0 = g * GRP
                    if c0 % (KSLAB // 128) == 0 and c0 > 0:
                        for h in range(HPC):
                            k_sb[h] = kp.tile([128, KSLAB], BF, tag="k",
                                              name=f"k{g}_{h}{r}")
                            nc.sync.dma_start(
                                k_sb[h][:],
                                kt_d[h * kslab_per_head + c0 // (KSLAB // 128)])
                    if c0 % VSLAB == 0 and c0 > 0:
                        v_sb = vp.tile([128, VSLAB * 2 * VW], BF, tag="v",
                                       name=f"v{g}{r}")
                        nc.sync.dma_start(v_sb[:], vb_d[c0 // VSLAB])

                    s_ps = pssp.tile([128, GRP * HPC * NQ], FP, tag="s",
                                     name=f"s{g}{r}")
                    for c2 in range(GRP):
                        koff = ((c0 + c2) % (KSLAB // 128)) * 128
                        for h in range(HPC):
                            nc.tensor.matmul(
                                s_ps[:, (c2 * HPC + h) * NQ:(c2 * HPC + h + 1) * NQ],
                                lhsT=k_sb[h][:, koff:koff + 128],
                                rhs=qT[h][:], start=True, stop=True)
                    p_sb = ptp.tile([128, GRP * HPC * NQ], BF, tag="pt",
                                    name=f"p{g}{r}")
                    nc.scalar.activation(p_sb[:], s_ps[:], AFT.Exp)
                    for c2 in range(GRP):
                        voff = ((c0 + c2) % VSLAB) * 2 * VW
                        for h in range(HPC):
                            nc.tensor.matmul(
                                accs[h][:],
                                lhsT=p_sb[:, (c2 * HPC + h) * NQ:(c2 * HPC + h + 1) * NQ],
                                rhs=v_sb[:, voff + h * VW:voff + (h + 1) * VW],
                                start=(g == 0 and c2 == 0), stop=False)
                for h in range(HPC):
                    nc.tensor.matmul(accs[h][:], lhsT=pnew[h][:], rhs=vnew[h][:],
                                     start=False, stop=True)

                # ---- normalize + transpose per head ----
                ut_tiles = []
                for h in range(HPC):
                    rec = smallp.tile([NQ, 1], FP, tag="rec", name=f"rec{h}{r}")
                    nc.vector.reciprocal(rec[:], accs[h][:, 128:129])
                    u_sb = smallp.tile([NQ, 128], BF, tag="u", name=f"u{h}{r}")
                    nc.scalar.activation(u_sb[:], accs[h][:, 0:128], AFT.Copy,
                                         scale=rec[:])
                    ut_ps = pmiscp.tile([128, NQ], BF, tag="misc", name=f"utps{h}{r}")
                    nc.tensor.transpose(ut_ps[:], u_sb[:], ident[0:NQ, 0:NQ])
                    ut_sb = smallp.tile([128, NQ], BF, tag="ut", name=f"ut{h}{r}")
                    nc.vector.tensor_copy(ut_sb[:], ut_ps[:])
                    ut_tiles.append(ut_sb)

                # ---- row-parallel output projection partial ----
                y_sb = smallp.tile([NQ, HID], FP, tag="y_sb", name=f"y{r}")
                for n in range(4):
                    y_ps = pgpp.tile([NQ, 512], FP, tag="gp", name=f"yps{n}{r}")
                    for h in range(HPC):
                        nc.tensor.matmul(y_ps[:], lhsT=ut_tiles[h][:],
                                         rhs=wp_sb[:, h * HID + n * 512:h * HID + (n + 1) * 512],
                                         start=(h == 0), stop=(h == HPC - 1))
                    nc.scalar.activation(y_sb[:, n * 512:(n + 1) * 512], y_ps[:],
                                         AFT.Copy)
                nc.sync.dma_start(out_d[:], y_sb[:])

            if loop is None:
                for rep in range(reps):
                    emit(f"r{rep}")
            else:
                with tc.For_i(0, loop, 1,
                              hint_engines=(mybir.EngineType.PE,)):
                    emit("rl")

    nc.compile()
    return nc


def _prep_shards(x, cached_k, cached_v, w_qkv, b_qkv, w_proj):
    scale = np.float32(1.0 / math.sqrt(D))
    x2d = np.asarray(x, np.float32).reshape(NQ, HID)
    xt_host = x2d.T.reshape(16, 128, NQ).transpose(1, 0, 2).reshape(128, XT_C)
    mask = np.zeros((128, MS_C), np.float32)
    mask[0:NQ] = np.kron(np.eye(B, dtype=np.float32), np.ones((S, S), np.float32))

    ck = np.asarray(cached_k, np.float32)
    cv = np.asarray(cached_v, np.float32)
    wq = np.asarray(w_qkv, np.float32)
    bq = np.asarray(b_qkv, np.float32)
    wp = np.asarray(w_proj, np.float32)

    in_maps = []
    for core in range(NCORES):
        h0 = HPC * core
        cols = slice(h0 * D, (h0 + HPC) * D)
        w_shard = np.concatenate(
            [wq[:, 0:HID][:, cols] * scale, wq[:, HID:2 * HID][:, cols],
             wq[:, 2 * HID:3 * HID][:, cols]], axis=1)          # [2048, 768]
        wq_nat = w_shard.reshape(16, 128, 768).transpose(1, 0, 2).reshape(128, WQ_C)
        wx_host = np.ascontiguousarray(
            np.concatenate([xt_host, wq_nat, mask], axis=1)).astype(NPBF)

        b_shard = np.concatenate(
            [bq[0:HID][cols] * scale, bq[HID:2 * HID][cols],
             bq[2 * HID:3 * HID][cols]])
        bqn_host = np.ascontiguousarray(b_shard.reshape(1, 768)).astype(NPBF)

        kt_slabs = []
        for h in (h0, h0 + 1):
            kt_h = ck[:, h, :].T                                 # [128, 8192]
            kt_slabs.append(kt_h.reshape(128, P // KSLAB, KSLAB).transpose(1, 0, 2))
        kt_host = np.ascontiguousarray(np.concatenate(kt_slabs, axis=0)).astype(NPBF)

        vb = np.empty((P, 2 * VW), np.float32)
        vb[:, 0:D] = cv[:, h0, :]
        vb[:, D] = 1.0
        vb[:, VW:VW + D] = cv[:, h0 + 1, :]
        vb[:, VW + D] = 1.0
        vb_host = np.ascontiguousarray(
            vb.reshape(NCHUNK // VSLAB, VSLAB, 128, 2 * VW)
              .transpose(0, 2, 1, 3).reshape(NCHUNK // VSLAB, 128, VSLAB * 2 * VW)
        ).astype(NPBF)

        wp_host = np.ascontiguousarray(
            np.concatenate([wp[(h0 + h) * D:(h0 + h + 1) * D, :]
                            for h in range(HPC)], axis=1)).astype(NPBF)

        in_maps.append({
            "wx": wx_host, "bqn": bqn_host,
            "kt": kt_host, "vb": vb_host, "wp": wp_host,
        })
    return in_maps


def kernel(**inputs):
    global _nc_cache
    x = np.asarray(inputs["x"], np.float32)
    b_proj = np.asarray(inputs["b_proj"], np.float32)
    in_maps = _prep_shards(
        x, inputs["cached_k"], inputs["cached_v"],
        inputs["w_qkv"], inputs["b_qkv"], inputs["w_proj"],
    )
    if _nc_cache is None:
        _nc_cache = _build_nc()
    res = run_bass_kernel_spmd(_nc_cache, in_maps, core_ids=list(range(NCORES)))
    y = np.zeros((NQ, HID), np.float64)
    for r in res.results:
        y += r["out"].astype(np.float64)
    y += b_proj.astype(np.float64)
    return y.astype(np.float32).reshape(B, S, HID)


# revision 11
# speedup vs baseline: 1.0940x; 1.0940x over previous
"""Trainium2 Bass kernel for nn_AttentionLayer (sparse_attention, 8-core head-parallel).

Reference computation (B=4, S=16, H=16, D=128, HID=2048, P=8192):
    qkv = x @ w_qkv + b_qkv ; split into q,k,v
    k_full = concat(cached_k broadcast over batch, new k)   # [B,H,P+S,D]
    out = softmax(q @ k_full^T / sqrt(D)) @ v_full
    y = out @ w_proj + b_proj

Sharding: tensor-parallel over heads. Each of the 8 cores owns 2 heads:
column-sharded w_qkv/b_qkv (its heads' q,k,v columns), the head slice of the
KV cache, and the row slice of w_proj. Each core emits a partial y
[64, 2048]; the unshard step sums the 8 partials and adds b_proj (row-parallel
linear with host-side reduction).

Device-side layout choices (prepared on host during the shard step):
  - TensorEngine-facing tensors are shipped/computed in bf16 (KV cache,
    weights, x, exp(scores)); all matmul accumulation, softmax input, the
    denominators and the output stay f32. Emulated end-to-end rel err vs the
    f32 reference is ~3e-3 (tolerance 2e-2).
  - x is passed pre-transposed k-major so it is directly the moving operand
    of the qkv projection; the projection is computed transposed
    (qkv^T = W_tile^T . x_tile, full 128 output partitions) so q^T/k^T/v^T
    come straight out of the bias activation with no extra transposes.
  - cached_k passed per head as K^T [D=128, P] in slab-contiguous form: each
    [128, 4096] bf16 slab is one contiguous 1MB DMA whose [128,128] slices
    are directly the stationary operand of the scores^T matmul.
  - cached_v passed with both heads interleaved [P, 129+129]: per-head 128
    value columns plus a constant ones column. Accumulating exp(scores^T)^T @
    [V | 1] yields the attention numerator AND softmax denominator in one
    matmul (scores are O(5) here, so exp needs no max-subtraction in f32).
  - 1/sqrt(D) is folded into the q columns of w_qkv/b_qkv.
  - New-token scores use a block-diagonal mask (queries attend only their own
    batch's 16 new keys), multiplied after exp -> exact zeros off-block.
  - scores^T for 4 chunks x 2 heads are packed into one [128, 512] PSUM bank
    so a single ACT instruction computes exp for all 8 score tiles.
  - The full 12.6MB input stream is resident in SBUF; every input DMA is
    issued up front so the HW DGE queues stay saturated with zero
    slot-recycling stalls.
"""

import math

import numpy as np
import ml_dtypes

import concourse.bass as bass
import concourse.mybir as mybir
import concourse.tile as tile
from concourse import bacc
from concourse.bass_utils import run_bass_kernel_spmd
from concourse.masks import make_identity

FP = mybir.dt.float32
BF = mybir.dt.bfloat16
NPBF = ml_dtypes.bfloat16
AFT = mybir.ActivationFunctionType

B, S, H, D = 4, 16, 16, 128
HID = H * D            # 2048
P = 8192               # cached prefix length
NQ = B * S             # 64 query tokens
NCORES = 8
HPC = H // NCORES      # heads per core = 2

NCHUNK = P // 128      # 64 cache chunks of 128 keys
GRP = 4                # chunks whose scores share one PSUM bank / one exp
NGRP = NCHUNK // GRP   # 16
KSLAB = 4096           # seq per K-slab DMA (32 chunks, 1MB bf16)
NKSLAB = P // KSLAB    # 2 slabs per head
VSLAB = 16             # chunks per V-slab DMA (1.03MB bf16)
NVSLAB = NCHUNK // VSLAB
VW = D + 1             # 129: V columns + ones column

_nc_cache = None


def _build_nc(reps=1, loop=None):
    nc = bacc.Bacc("TRN2", target_bir_lowering=False, debug=False,
                   num_devices=NCORES)

    xt_d = nc.declare_dram_parameter("xt", [128, 16 * NQ], BF, isOutput=False)
    wqkv_d = nc.declare_dram_parameter("wqkv", [128, 6 * 2048], BF, isOutput=False)
    bqkv_d = nc.declare_dram_parameter("bqkv", [128, 6], FP, isOutput=False)
    mask_d = nc.declare_dram_parameter("mask", [NQ, NQ], BF, isOutput=False)
    kt_d = nc.declare_dram_parameter("kt", [HPC * NKSLAB, 128, KSLAB], BF, isOutput=False)
    vb_d = nc.declare_dram_parameter("vb", [NVSLAB, 128, VSLAB * 2 * VW], BF, isOutput=False)
    wp_d = nc.declare_dram_parameter("wp", [128, HPC * HID], BF, isOutput=False)
    out_d = nc.declare_dram_parameter("out", [NQ, HID], FP, isOutput=True)

    with tile.TileContext(nc) as tc:
        with (
            tc.tile_pool(name="const", bufs=1) as constp,
            tc.tile_pool(name="wqkv", bufs=3) as wqp,
            tc.tile_pool(name="wproj", bufs=1) as wpp,
            tc.tile_pool(name="kslab", bufs=2 * NKSLAB) as kp,
            tc.tile_pool(name="vslab", bufs=NVSLAB) as vp,
            tc.tile_pool(name="pt", bufs=4) as ptp,
            tc.tile_pool(name="small", bufs=4) as smallp,
            tc.tile_pool(name="ps_s", bufs=3, space="PSUM") as pssp,
            tc.tile_pool(name="ps_acc", bufs=2, space="PSUM") as paccp,
            tc.tile_pool(name="ps_gp", bufs=2, space="PSUM") as pgpp,
            tc.tile_pool(name="ps_misc", bufs=1, space="PSUM") as pmiscp,
        ):
            ident = constp.tile([128, 128], BF, tag="ident")
            make_identity(nc, ident[:])

            def emit(r):
                # ---- the whole input stream, issued up front ----
                xt = constp.tile([128, 16 * NQ], BF, tag="xt", name=f"xt{r}")
                nc.sync.dma_start(xt[:], xt_d[:])
                bq = constp.tile([128, 6], FP, tag="bq", name=f"bq{r}")
                nc.sync.dma_start(bq[:], bqkv_d[:])
                msk = constp.tile([NQ, NQ], BF, tag="msk", name=f"msk{r}")
                nc.sync.dma_start(msk[:], mask_d[:])
                wq_tiles = []
                for w2 in range(3):
                    t_ = wqp.tile([128, 4096], BF, tag="wqkv", name=f"wq{w2}{r}")
                    nc.sync.dma_start(t_[:], wqkv_d[:, w2 * 4096:(w2 + 1) * 4096])
                    wq_tiles.append(t_)
                k_tiles = []           # [h*NKSLAB + s]
                for h in range(HPC):
                    for s_ in range(NKSLAB):
                        t_ = kp.tile([128, KSLAB], BF, tag="k", name=f"k{h}_{s_}{r}")
                        nc.sync.dma_start(t_[:], kt_d[h * NKSLAB + s_])
                        k_tiles.append(t_)
                v_tiles = []
                for s_ in range(NVSLAB):
                    t_ = vp.tile([128, VSLAB * 2 * VW], BF, tag="v", name=f"v{s_}{r}")
                    nc.sync.dma_start(t_[:], vb_d[s_])
                    v_tiles.append(t_)
                wp_sb = wpp.tile([128, HPC * HID], BF, tag="wp", name=f"wp{r}")
                nc.sync.dma_start(wp_sb[:], wp_d[:])

                # ---- qkv projection: qkvT[m] = (x @ Wm + bm)^T  [128, 64] bf16
                # m = 0,1: q^T per head (scale pre-folded); 2,3: k^T; 4,5: v^T
                qkvT = []
                for m in range(6):
                    ps = pgpp.tile([128, NQ], FP, tag="gp", name=f"qkvps{m}{r}")
                    for t in range(16):
                        nc.tensor.matmul(
                            ps[:],
                            lhsT=wq_tiles[m // 2][:, (m % 2) * 2048 + t * 128:(m % 2) * 2048 + (t + 1) * 128],
                            rhs=xt[:, t * NQ:(t + 1) * NQ],
                            start=(t == 0), stop=(t == 15))
                    sb = constp.tile([128, NQ], BF, tag=f"qkvT{m}", name=f"qkvT{m}{r}")
                    nc.scalar.activation(sb[:], ps[:], AFT.Identity, bias=bq[:, m:m + 1])
                    qkvT.append(sb)

                # ---- new-token attention pieces (tiny) ----
                vnew = []
                pnew = []
                for h in range(HPC):
                    vt_ps = pmiscp.tile([NQ, 128], BF, tag="misc", name=f"vtps{h}{r}")
                    nc.tensor.transpose(vt_ps[:], qkvT[4 + h][:], ident[:])
                    vn = constp.tile([NQ, VW], BF, tag=f"vnew{h}", name=f"vnew{h}{r}")
                    nc.scalar.activation(vn[:, 0:128], vt_ps[:], AFT.Copy)
                    nc.vector.memset(vn[:, 128:129], 1.0)
                    vnew.append(vn)

                    sn_ps = pmiscp.tile([NQ, NQ], FP, tag="misc", name=f"snps{h}{r}")
                    nc.tensor.matmul(sn_ps[:], lhsT=qkvT[2 + h][:], rhs=qkvT[h][:],
                                     start=True, stop=True)
                    pn = constp.tile([NQ, NQ], BF, tag=f"pn{h}", name=f"pn{h}{r}")
                    nc.scalar.activation(pn[:], sn_ps[:], AFT.Exp)
                    pnm = constp.tile([NQ, NQ], BF, tag=f"pnm{h}", name=f"pnm{h}{r}")
                    nc.vector.tensor_mul(pnm[:], pn[:], msk[:])
                    pnew.append(pnm)

                # ---- main cache sweep, both heads interleaved ----
                accs = [paccp.tile([NQ, VW], FP, tag="acc", name=f"acc{i}{r}")
                        for i in range(HPC)]
                for g in range(NGRP):
                    c0 = g * GRP
                    s_ps = pssp.tile([128, GRP * HPC * NQ], FP, tag="s",
                                     name=f"s{g}{r}")
                    for c2 in range(GRP):
                        c = c0 + c2
                        kslab = c // (KSLAB // 128)
                        koff = (c % (KSLAB // 128)) * 128
                        for h in range(HPC):
                            nc.tensor.matmul(
                                s_ps[:, (c2 * HPC + h) * NQ:(c2 * HPC + h + 1) * NQ],
                                lhsT=k_tiles[h * NKSLAB + kslab][:, koff:koff + 128],
                                rhs=qkvT[h][:], start=True, stop=True)
                    p_sb = ptp.tile([128, GRP * HPC * NQ], BF, tag="pt",
                                    name=f"p{g}{r}")
                    nc.scalar.activation(p_sb[:], s_ps[:], AFT.Exp)
                    for c2 in range(GRP):
                        c = c0 + c2
                        v_sb = v_tiles[c // VSLAB]
                        voff = (c % VSLAB) * 2 * VW
                        for h in range(HPC):
                            nc.tensor.matmul(
                                accs[h][:],
                                lhsT=p_sb[:, (c2 * HPC + h) * NQ:(c2 * HPC + h + 1) * NQ],
                                rhs=v_sb[:, voff + h * VW:voff + (h + 1) * VW],
                                start=(g == 0 and c2 == 0), stop=False)
                for h in range(HPC):
                    nc.tensor.matmul(accs[h][:], lhsT=pnew[h][:], rhs=vnew[h][:],
                                     start=False, stop=True)

                # ---- normalize + transpose per head ----
                ut_tiles = []
                for h in range(HPC):
                    rec = smallp.tile([NQ, 1], FP, tag="rec", name=f"rec{h}{r}")
                    nc.vector.reciprocal(rec[:], accs[h][:, 128:129])
                    u_sb = smallp.tile([NQ, 128], BF, tag="u", name=f"u{h}{r}")
                    nc.scalar.activation(u_sb[:], accs[h][:, 0:128], AFT.Copy,
                                         scale=rec[:])
                    ut_ps = pmiscp.tile([128, NQ], BF, tag="misc", name=f"utps{h}{r}")
                    nc.tensor.transpose(ut_ps[:], u_sb[:], ident[0:NQ, 0:NQ])
                    ut_sb = smallp.tile([128, NQ], BF, tag="ut", name=f"ut{h}{r}")
                    nc.vector.tensor_copy(ut_sb[:], ut_ps[:])
                    ut_tiles.append(ut_sb)

                # ---- row-parallel output projection partial ----
                y_sb = smallp.tile([NQ, HID], FP, tag="y_sb", name=f"y{r}")
                for n in range(4):
                    y_ps = pgpp.tile([NQ, 512], FP, tag="gp", name=f"yps{n}{r}")
                    for h in range(HPC):
                        nc.tensor.matmul(y_ps[:], lhsT=ut_tiles[h][:],
                                         rhs=wp_sb[:, h * HID + n * 512:h * HID + (n + 1) * 512],
                                         start=(h == 0), stop=(h == HPC - 1))
                    nc.scalar.activation(y_sb[:, n * 512:(n + 1) * 512], y_ps[:],
                                         AFT.Copy)
                nc.sync.dma_start(out_d[:], y_sb[:])

            if loop is None:
                for rep in range(reps):
                    emit(f"r{rep}")
            else:
                with tc.For_i(0, loop, 1,
                              hint_engines=(mybir.EngineType.PE,)):
                    emit("rl")

    nc.compile()
    return nc


def _prep_shards(x, cached_k, cached_v, w_qkv, b_qkv, w_proj):
    scale = np.float32(1.0 / math.sqrt(D))
    x2d = np.asarray(x, np.float32).reshape(NQ, HID)
    xt_host = np.ascontiguousarray(
        x2d.T.reshape(16, 128, NQ).transpose(1, 0, 2).reshape(128, 16 * NQ)
    ).astype(NPBF)
    mask = np.ascontiguousarray(
        np.kron(np.eye(B, dtype=np.float32), np.ones((S, S), np.float32))
    ).astype(NPBF)

    ck = np.asarray(cached_k, np.float32)
    cv = np.asarray(cached_v, np.float32)
    wq = np.asarray(w_qkv, np.float32)
    bq = np.asarray(b_qkv, np.float32)
    wp = np.asarray(w_proj, np.float32)

    in_maps = []
    for core in range(NCORES):
        h0 = HPC * core
        cols = slice(h0 * D, (h0 + HPC) * D)
        w_shard = np.concatenate(
            [wq[:, 0:HID][:, cols] * scale, wq[:, HID:2 * HID][:, cols],
             wq[:, 2 * HID:3 * HID][:, cols]], axis=1)          # [2048, 768]
        wqkv_host = np.ascontiguousarray(
            w_shard.reshape(16, 128, 6, 128).transpose(1, 2, 0, 3).reshape(128, 6 * 2048)
        ).astype(NPBF)
        b_shard = np.concatenate(
            [bq[0:HID][cols] * scale, bq[HID:2 * HID][cols], bq[2 * HID:3 * HID][cols]])
        bqkv_host = np.ascontiguousarray(b_shard.reshape(6, 128).T)

        kt_slabs = []
        for h in (h0, h0 + 1):
            kt_h = ck[:, h, :].T                                 # [128, 8192]
            kt_slabs.append(kt_h.reshape(128, NKSLAB, KSLAB).transpose(1, 0, 2))
        kt_host = np.ascontiguousarray(np.concatenate(kt_slabs, axis=0)).astype(NPBF)

        vb = np.empty((P, 2 * VW), np.float32)
        vb[:, 0:D] = cv[:, h0, :]
        vb[:, D] = 1.0
        vb[:, VW:VW + D] = cv[:, h0 + 1, :]
        vb[:, VW + D] = 1.0
        vb_host = np.ascontiguousarray(
            vb.reshape(NVSLAB, VSLAB, 128, 2 * VW)
              .transpose(0, 2, 1, 3).reshape(NVSLAB, 128, VSLAB * 2 * VW)
        ).astype(NPBF)

        wp_host = np.ascontiguousarray(
            np.concatenate([wp[(h0 + h) * D:(h0 + h + 1) * D, :]
                            for h in range(HPC)], axis=1)).astype(NPBF)

        in_maps.append({
            "xt": xt_host, "wqkv": wqkv_host, "bqkv": bqkv_host, "mask": mask,
            "kt": kt_host, "vb": vb_host, "wp": wp_host,
        })
    return in_maps


def kernel(**inputs):
    global _nc_cache
    x = np.asarray(inputs["x"], np.float32)
    b_proj = np.asarray(inputs["b_proj"], np.float32)
    in_maps = _prep_shards(
        x, inputs["cached_k"], inputs["cached_v"],
        inputs["w_qkv"], inputs["b_qkv"], inputs["w_proj"],
    )
    if _nc_cache is None:
        _nc_cache = _build_nc()
    res = run_bass_kernel_spmd(_nc_cache, in_maps, core_ids=list(range(NCORES)))
    y = np.zeros((NQ, HID), np.float64)
    for r in res.results:
        y += r["out"].astype(np.float64)
    y += b_proj.astype(np.float64)
    return y.astype(np.float32).reshape(B, S, HID)
